# revision 19
# baseline (speedup 1.0000x reference)
"""Trainium2 Bass kernel for GQA attention (B=2, S=2048, HID=2048, 16 q-heads,
4 kv-heads, HD=128, RoPE, softmax, output projection).

Sharding: 8 cores = (2 batches) x (4 kv groups of 4 q-heads). Each core owns one
batch, 4 q heads + their kv head, and the matching 512-row slice of Wo
(Megatron row-parallel); the host sums the 4 bf16 partials per batch.

Design (vs the fp32r baseline this started from):
- All matmuls bf16: fp32r streams 253ns per 512-row matmul on HW vs bf16's
  217.6ns (serialized 4-byte weight loads). fp8-DoubleRow measured only 2x
  bf16 flops (not the cost model's 4x) and e4m3's ~2.4% rms error per tensor
  blows the 2e-2 tolerance in every uncompensated placement (verified by CPU
  simulation), so bf16 is the accuracy/speed optimum here.
- Software-conveyor pipeline keeping the in-order PE stream dependency-free:
  per head-phase the PE runs scores(h) interleaved with attnV(h-1) (whose
  exps completed a full phase earlier), a slice of the previous block's
  output projection, then den(h-1). Q projections for a block run inside its
  first head-phase, interleaved with the previous block's last-head attnV.
- Denominators: incremental DVE pair-sums of exp tiles during the jp loop,
  an all-ones [128,128] matmul for the cross-partition sum, fast approximate
  reciprocal; normalization folded into the osb write (row-scaled softmax
  commutes with the row-parallel output projection).
- DMA: host pre-chunks x/weights so every DMA line is 4KB contiguous;
  x on the sync queue, weights on the scalar queue; late-needed weights
  (wq/wo/cos/sin) are enqueued behind stage-A scalar instructions so the x
  stream gets full HBM bandwidth at the start; outputs leave as full
  [128, 2048] bf16 row slabs alternating between both queues.
- The last i-block is split into two 256-wide half-blocks so its output
  projection overlaps attention instead of serializing into the drain.
"""

import sys
import types

sys.path.insert(0, "/opt/trn_rl_repo")

import numpy as np
import ml_dtypes

B, S, HID = 2, 2048, 2048
NH, NKV, HD = 16, 4, 128
GROUPS = NH // NKV          # q heads per kv head == heads per core
ROPE_THETA = 10000.0
P = 128                     # SBUF partitions
SB = 512                    # s-block (matmul moving dim / psum bank)
N_CORES = 8
KC = HID // P               # 16 contraction chunks
JC = S // P                 # 16 key chunks
NB = S // SB                # 4 s/i blocks
NHC = GROUPS                # 4 heads per core

BF16NP = ml_dtypes.bfloat16

_built = None


def _install_ntff_hook():
    """antenv.axon_hooks is missing from the agent image, which silently
    disables trace=True; recreate it and register the ctypes NTFF hook."""
    if "antenv.axon_hooks" in sys.modules:
        return
    m = types.ModuleType("antenv.axon_hooks")
    m._hook = None
    m.set_axon_ntff_profile_hook = lambda h: setattr(m, "_hook", h)
    m.get_axon_ntff_profile_hook = lambda: m._hook
    sys.modules["antenv.axon_hooks"] = m
    try:
        import antenv

        antenv.axon_hooks = m
    except ImportError:
        pass
    try:
        sys.path.insert(0, "/root/.axon_site/trn_agent_boot")
        from trn_boot import _ntff_profile_via_ctypes

        hook = _ntff_profile_via_ctypes("/opt/axon/libaxon_pjrt.so")
        if hook is not None:
            m.set_axon_ntff_profile_hook(hook)
    except Exception:
        pass


_install_ntff_hook()


def rope_tables():
    """cos table and sign-folded sin table in [HD, S] (transposed) layout.

    sin_signed[d] = -sin for d < HD/2, +sin for d >= HD/2, so RoPE becomes
    out = q * cos + shifted(q) * sin_signed with shifted(q) a partition-half
    swap.
    """
    half = HD // 2
    inv_freq = 1.0 / (ROPE_THETA ** (np.arange(0, HD, 2, dtype=np.float64) / HD))
    t = np.arange(S, dtype=np.float64)
    freqs = np.outer(t, inv_freq)                      # [S, 64]
    emb = np.concatenate([freqs, freqs], axis=-1)      # [S, 128]
    cos_t = np.cos(emb).T.astype(np.float32).copy()    # [128, S]
    sin = np.sin(emb).T.astype(np.float32)
    sin_signed = sin.copy()
    sin_signed[:half] *= -1.0
    return cos_t, np.ascontiguousarray(sin_signed)


def build_bass():
    import concourse.mybir as mybir
    from concourse import bacc
    from concourse.tile import TileContext

    F32 = mybir.dt.float32
    BF16 = mybir.dt.bfloat16
    EXP = mybir.ActivationFunctionType.Exp
    MUL = mybir.AluOpType.mult

    scale = 1.0 / float(np.sqrt(HD))

    nc = bacc.Bacc("TRN2")

    # host-pre-chunked layouts: DMA inner lines are 4KB contiguous
    xT_d = nc.dram_tensor("xT", [P, NB, KC, SB], BF16, kind="ExternalInput")
    wq_d = nc.dram_tensor("wq", [P, KC, NHC * HD], BF16, kind="ExternalInput")
    wk_d = nc.dram_tensor("wk", [P, KC, HD], BF16, kind="ExternalInput")
    wv_d = nc.dram_tensor("wv", [P, KC, HD], BF16, kind="ExternalInput")
    wo_d = nc.dram_tensor("wo", [P, NHC, HID], BF16, kind="ExternalInput")
    cos_d = nc.dram_tensor("cos_t", [P, S], BF16, kind="ExternalInput")
    sin_d = nc.dram_tensor("sin_t", [P, S], BF16, kind="ExternalInput")
    ident_d = nc.dram_tensor("ident", [P, P], BF16, kind="ExternalInput")
    ones_d = nc.dram_tensor("ones_mat", [P, P], BF16, kind="ExternalInput")
    out_d = nc.dram_tensor("out", [S, HID], BF16, kind="ExternalOutput")

    with TileContext(nc) as tc:
        with (
            tc.tile_pool(name="const", bufs=1) as cpool,
            tc.tile_pool(name="xt", bufs=1) as xtp,
            tc.tile_pool(name="w", bufs=1) as wp,
            tc.tile_pool(name="kv", bufs=1) as kvp,
            tc.tile_pool(name="qt", bufs=2) as qtp,
            tc.tile_pool(name="et", bufs=2) as etp,
            tc.tile_pool(name="sm", bufs=2) as smp,
            tc.tile_pool(name="osb", bufs=2) as osbp,
            tc.tile_pool(name="po", bufs=3) as pop,
            tc.tile_pool(name="vt", bufs=2) as vtp,
        ):
            # ---- DMA priority order. Aggregate HBM intake is ~300GB/s per
            # core (all 8 cores pull at once), so bytes must arrive in the
            # exact order stage A consumes them:
            #   scalar q: wk/wv (first KV matmul ~3us), then cos/sin (first
            #             rope, DVE-side slack), wo NOT here (enqueued in
            #             stage B; first po needs it ~40us after stage A)
            #   sync+gpsimd qs: x block0, then wq (q0 runs right after KV
            #             block0), then x blocks 1-3
            wk_sb = wp.tile([P, KC, HD], BF16, tag="wk")
            wv_sb = wp.tile([P, KC, HD], BF16, tag="wv")
            for c in range(0, KC, 2):
                nc.scalar.dma_start(wk_sb[:, c : c + 2, :], wk_d[:, c : c + 2, :])
                nc.scalar.dma_start(wv_sb[:, c : c + 2, :], wv_d[:, c : c + 2, :])
            ident_bf = cpool.tile([P, P], BF16, tag="ident")
            nc.scalar.dma_start(ident_bf[:], ident_d[:, :])
            ones_bf = cpool.tile([P, P], BF16, tag="ones")
            nc.scalar.dma_start(ones_bf[:], ones_d[:, :])

            xts = []
            for sb in range(NB):
                xt_blk = xtp.tile([P, KC, SB], BF16, tag=f"xt{sb}")
                xts.append(xt_blk)

            wq_sb = wp.tile([P, KC, NHC * HD], BF16, tag="wq")
            wo_sb = wp.tile([P, NHC, HID], BF16, tag="wo")
            cos_t = cpool.tile([P, S], BF16, tag="cos")
            sin_t = cpool.tile([P, S], BF16, tag="sin")

            # x block 0: single chunks, 2 of 3 on gpsimd (its SWDGE queue
            # sustains ~2x the sync HWDGE rate under contention)
            for c in range(KC):
                eng = nc.sync if c % 3 == 0 else nc.gpsimd
                eng.dma_start(xts[0][:, c : c + 1, :], xT_d[:, 0, c : c + 1, :])
            # wq next at 2-chunk granularity (q0 filler chunks start
            # consuming wq kc=0.. from ~13us); cos/sin are enqueued later,
            # behind block-0's vtmp copy on the scalar engine
            for ci, c in enumerate(range(0, KC, 2)):
                eng = nc.sync if ci % 3 == 0 else nc.gpsimd
                eng.dma_start(wq_sb[:, c : c + 2, :], wq_d[:, c : c + 2, :])
            # x blocks 1-3: chunk pairs, same 1:2 sync:gpsimd split
            for sb in range(1, NB):
                for ci, c in enumerate(range(0, KC, 2)):
                    eng = nc.sync if ci % 3 == 0 else nc.gpsimd
                    eng.dma_start(
                        xts[sb][:, c : c + 2, :],
                        xT_d[:, sb, c : c + 2, :],
                    )

            # per-block K/V tiles: scores/attnV of early key blocks must not
            # carry a (whole-tile) dependency on the LAST block's rope/copy
            kTs = [
                kvp.tile([P, SB], BF16, tag=f"kT{sb}", name=f"kT{sb}")
                for sb in range(NB)
            ]
            vnats = [
                kvp.tile([P, SB // P, HD], BF16, tag=f"vnat{sb}",
                         name=f"vnat{sb}")
                for sb in range(NB)
            ]

            def rope(dst, src_ps, gs, w, eng):
                # dst = src * cos + shifted_halves(src) * sin_signed
                h2 = HD // 2
                tmp = smp.tile([P, SB], BF16, tag="ropetmp")
                eng.tensor_tensor(
                    dst, src_ps[:, 0:w], cos_t[:, gs : gs + w], MUL
                )
                eng.tensor_tensor(
                    tmp[0:h2, 0:w], src_ps[h2:P, 0:w], sin_t[0:h2, gs : gs + w],
                    MUL,
                )
                eng.tensor_tensor(
                    tmp[h2:P, 0:w], src_ps[0:h2, 0:w], sin_t[h2:P, gs : gs + w],
                    MUL,
                )
                eng.tensor_add(dst, dst, tmp[:, 0:w])

            # ================= Stage A =================
            # Consumption matches the DMA priority order exactly:
            # KV(block0) -> Q(block0, head-major; wq streams in under it) ->
            # KV(block1..3) paced by the two x queues. The in-order PE then
            # never runs more than slightly ahead of the HBM intake.
            qT0 = qtp.tile([P, NHC, SB], BF16, tag="qT")
            with (
                tc.tile_pool(name="pskv", bufs=5, space="PSUM") as pskv,
                tc.tile_pool(name="pstp", bufs=2, space="PSUM") as pstp,
            ):
                def v_transpose(sb, vtmp):
                    tps = pstp.tile([P, SB], BF16, tag="tps")
                    for t in range(SB // P):
                        nc.tensor.transpose(
                            tps[:, t * P : (t + 1) * P],
                            vtmp[:, t * P : (t + 1) * P],
                            ident_bf[:],
                        )
                    nc.scalar.copy(
                        vnats[sb][:, :, :],
                        tps[:].rearrange("p (a b) -> p a b", a=SB // P),
                    )

                # Block-0 Q chunks are the PE's filler work: one (or two, in
                # block 3) is emitted ahead of each KV chunk pair of blocks
                # 1-3, so whenever the x stream falls behind, the in-order
                # PE still has resident-data work queued. wq arrives early
                # (right after x block 0 on both big queues).
                q0_state = [None, 0]   # open q_ps, next flat chunk index

                def q0_chunk():
                    qh, kc = divmod(q0_state[1], KC)
                    if qh >= NHC:
                        return
                    q0_state[1] += 1
                    if kc == 0:
                        q0_ps = pskv.tile([P, SB], F32, tag="kv")
                        q0_state[0] = q0_ps
                    nc.tensor.matmul(
                        q0_state[0][:],
                        wq_sb[:, kc, qh * HD : (qh + 1) * HD],
                        xts[0][:, kc, :],
                        start=(kc == 0), stop=(kc == KC - 1),
                    )
                    if kc == KC - 1:
                        rope(qT0[:, qh, :], q0_state[0], 0, SB, nc.vector)

                def kv_block_interleaved(sb, nq):
                    xt = xts[sb]
                    k_ps = pskv.tile([P, SB], F32, tag="kv")
                    v_ps = pskv.tile([P, SB], F32, tag="kv")
                    for kc in range(KC):
                        for _ in range(nq):
                            q0_chunk()
                        nc.tensor.matmul(
                            k_ps[:], wk_sb[:, kc, :], xt[:, kc, :],
                            start=(kc == 0), stop=(kc == KC - 1),
                        )
                        nc.tensor.matmul(
                            v_ps[:], wv_sb[:, kc, :], xt[:, kc, :],
                            start=(kc == 0), stop=(kc == KC - 1),
                        )
                    vtmp = vtp.tile([P, SB], BF16, tag="vtmp")
                    nc.scalar.copy(vtmp[:], v_ps[:])
                    if sb == 0:
                        nc.scalar.dma_start(cos_t[:], cos_d[:, :])
                        nc.scalar.dma_start(sin_t[:], sin_d[:, :])
                    rope(kTs[sb][:, 0:SB], k_ps, sb * SB, SB, nc.vector)
                    v_transpose(sb, vtmp)

                for sb in range(NB):
                    kv_block_interleaved(sb, 0 if sb == 0 else 1 if sb < 3 else 2)

            # ====== Stage B conveyor ======
            # Per head-phase the PE runs: scores(h) pairs interleaved with
            # attnV(h-1) pairs, then a few outproj groups of the previous
            # block, then den(h-1). attnV of the last head of a block runs
            # inside the next block's Q phase. This keeps every cross-engine
            # consumer (exp on ACT, esum/recip on DVE) a full phase ahead of
            # the in-order PE instruction that needs it.
            with (
                tc.tile_pool(name="psS", bufs=2, space="PSUM") as psS,
                tc.tile_pool(name="psAcc", bufs=2, space="PSUM") as psAcc,
                tc.tile_pool(name="psM", bufs=2, space="PSUM") as psM,
            ):
                HB = SB // 2   # half-block width for the tail blocks
                BLOCKS = [(0, SB), (SB, SB), (2 * SB, SB), (3 * SB, HB),
                          (3 * SB + HB, HB)]

                class Head:
                    __slots__ = (
                        "h", "e_t", "es4", "o_ps", "osb_t", "w", "gi", "recip"
                    )

                def emit_scores_pair(st, qT_t, jp):
                    jc0 = 2 * jp
                    w = st.w
                    kb, kl = divmod(jc0, SB // P)
                    s_ps = psS.tile([P, 2, SB], F32, tag="s")
                    nc.tensor.matmul(
                        s_ps[:, 0, 0:w], kTs[kb][:, kl * P : (kl + 1) * P],
                        qT_t[:, st.h, 0:w], start=True, stop=True,
                    )
                    nc.tensor.matmul(
                        s_ps[:, 1, 0:w], kTs[kb][:, (kl + 1) * P : (kl + 2) * P],
                        qT_t[:, st.h, 0:w], start=True, stop=True,
                    )
                    nc.scalar.activation(
                        st.e_t[:, jc0 : jc0 + 2, 0:w], s_ps[:, :, 0:w], EXP,
                        scale=scale,
                    )
                    if jp == 0:
                        nc.vector.tensor_copy(
                            st.es4[:, :, 0:w], st.e_t[:, 0:2, 0:w]
                        )
                    else:
                        nc.vector.tensor_add(
                            st.es4[:, :, 0:w], st.es4[:, :, 0:w],
                            st.e_t[:, jc0 : jc0 + 2, 0:w],
                        )

                def emit_attnv_pair(st, jp):
                    jc0, jc1 = 2 * jp, 2 * jp + 1
                    w = st.w
                    vb, vl = divmod(jc0, SB // P)
                    if jp == 0:
                        st.o_ps = psAcc.tile([P, SB], F32, tag="acc")
                    nc.tensor.matmul(
                        st.o_ps[:, 0:w], vnats[vb][:, vl, :],
                        st.e_t[:, jc0, 0:w],
                        start=(jc0 == 0), stop=False,
                    )
                    nc.tensor.matmul(
                        st.o_ps[:, 0:w], vnats[vb][:, vl + 1, :],
                        st.e_t[:, jc1, 0:w],
                        start=False, stop=(jc1 == JC - 1),
                    )

                def emit_den_pre(st):
                    # denominator chain up to the reciprocal; only needs the
                    # exp tiles (es4), not o_ps, so it can run before attnV
                    # of st finishes.
                    w = st.w
                    esum = smp.tile([P, SB], BF16, tag="esum")
                    nc.vector.tensor_add(
                        esum[:, 0:w], st.es4[:, 0, 0:w], st.es4[:, 1, 0:w]
                    )
                    den_ps = psM.tile([P, SB], F32, tag="m")
                    nc.tensor.matmul(
                        den_ps[:, 0:w], ones_bf[:], esum[:, 0:w],
                        start=True, stop=True,
                    )
                    st.recip = smp.tile([P, SB], F32, tag="recip")
                    scratch = smp.tile([P, SB], F32, tag="scratch")
                    nc.vector.reciprocal_approx_accurate(
                        st.recip[:, 0:w], den_ps[:, 0:w], scratch[:, 0:w]
                    )

                def emit_den_post(st):
                    w = st.w
                    nc.vector.tensor_tensor(
                        st.osb_t[:, st.h, 0:w], st.o_ps[:, 0:w],
                        st.recip[:, 0:w], MUL,
                    )

                def emit_den(st):
                    emit_den_pre(st)
                    emit_den_post(st)

                class PoQ:
                    __slots__ = ("gi", "osb_t", "idx", "row", "n")

                def emit_po_group(po):
                    po_ic, po_oc = divmod(po.idx, HID // SB)
                    po.idx += 1
                    po_ps = psM.tile([P, SB], F32, tag="m")
                    for po_h in range(NHC):
                        nc.tensor.matmul(
                            po_ps[:],
                            po.osb_t[:, po_h, po_ic * P : (po_ic + 1) * P],
                            wo_sb[:, po_h, po_oc * SB : (po_oc + 1) * SB],
                            start=(po_h == 0), stop=(po_h == NHC - 1),
                        )
                    if po_oc == 0:
                        po.row = pop.tile([P, HID], BF16, tag="po")
                    # PSUM->SBUF cast alternates DVE/ACT so the psM
                    # bank-recycle path isn't serialized behind one queue
                    # (gpsimd cannot read PSUM on TRN2)
                    if po_oc % 2 == 0:
                        nc.vector.tensor_copy(
                            po.row[:, po_oc * SB : (po_oc + 1) * SB], po_ps[:]
                        )
                    else:
                        nc.scalar.copy(
                            po.row[:, po_oc * SB : (po_oc + 1) * SB], po_ps[:]
                        )
                    eng = nc.sync if po_ic % 2 == 0 else nc.scalar
                    if po.gi >= 3 * SB:
                        # tail half-blocks: DMA each quarter as soon as its
                        # copy lands, so only 128KB trails the last matmul
                        eng.dma_start(
                            out_d[po.gi + po_ic * P : po.gi + (po_ic + 1) * P,
                                  po_oc * SB : (po_oc + 1) * SB],
                            po.row[:, po_oc * SB : (po_oc + 1) * SB],
                        )
                    elif po_oc == HID // SB - 1:
                        # full 128-row slab: one DMA with 4KB lines
                        eng.dma_start(
                            out_d[po.gi + po_ic * P : po.gi + (po_ic + 1) * P, :],
                            po.row[:],
                        )

                def po_drain(po, share):
                    # emit roughly `share` of the remaining outproj groups
                    n = min(share, po.n - po.idx)
                    for _ in range(n):
                        emit_po_group(po)

                prev = None   # Head awaiting attnV + den
                po_q = None   # outproj of previous block
                for bi, (gi, w) in enumerate(BLOCKS):
                    xt = xts[gi // SB]
                    off = gi % SB
                    last_block = bi == len(BLOCKS) - 1
                    if po_q is None or po_q.n == 16:
                        spread = [5, 4, 4, 3]
                    elif last_block:
                        # reserve 2 groups as PE cover for the final den
                        # chain in the drain
                        spread = [2, 2, 2, 0]
                    else:
                        spread = [2, 2, 2, 2]

                    if bi == 0:
                        qT_t = qT0
                    else:
                        qT_t = qtp.tile([P, NHC, SB], BF16, tag="qT")
                    osb_t = osbp.tile([P, NHC, SB], BF16, tag="osb")
                    for h in range(NHC):
                        last_head = bi == len(BLOCKS) - 1 and h == NHC - 1
                        st = Head()
                        st.h = h
                        st.osb_t = osb_t
                        st.o_ps = None
                        st.w = w
                        st.gi = gi
                        st.e_t = etp.tile([P, JC, SB], BF16, tag="E")
                        st.es4 = smp.tile([P, 2, SB], BF16, tag="es4")
                        if h == 0 and bi > 0:
                            # Q projections for the whole block, interleaved
                            # with the previous block's last-head attnV; its
                            # den lands before the po groups need its osb.
                            for qh in range(NHC):
                                q_ps = psM.tile([P, SB], F32, tag="m")
                                for kc in range(KC):
                                    nc.tensor.matmul(
                                        q_ps[:, 0:w],
                                        wq_sb[:, kc, qh * HD : (qh + 1) * HD],
                                        xt[:, kc, off : off + w],
                                        start=(kc == 0), stop=(kc == KC - 1),
                                    )
                                rope(qT_t[:, qh, 0:w], q_ps, gi, w,
                                     nc.vector)
                                if prev is not None:
                                    emit_attnv_pair(prev, 2 * qh)
                                    emit_attnv_pair(prev, 2 * qh + 1)
                            if prev is not None:
                                emit_den(prev)
                                prev = None
                        for jp in range(JC // 2):
                            emit_scores_pair(st, qT_t, jp)
                            if prev is not None:
                                emit_attnv_pair(prev, jp)
                            if last_head and jp >= 2:
                                # inline own attnV at lag 2 so the drain is
                                # short (exp(jp-2) is long done)
                                emit_attnv_pair(st, jp - 2)
                        if po_q is not None:
                            po_drain(po_q, spread[h])
                        if prev is not None:
                            emit_den(prev)
                        if bi == 0:
                            # wo queue entry per block-0 phase: transfers run
                            # ~55-75us, first po_drain needs wo at ~95us
                            nc.scalar.dma_start(wo_sb[:, h, :], wo_d[:, h, :])
                        prev = st
                    po_prev_q = po_q
                    po_q = PoQ()
                    po_q.gi = gi
                    po_q.osb_t = osb_t
                    po_q.idx = 0
                    po_q.n = (w // P) * (HID // SB)

                # ---- drain: finish final head, then its block's outproj ----
                # order: last attnV pairs, then the 2 reserved po groups of
                # the previous half-block cover the exp->es4->esum latency of
                # the final den chain; den_post waits only on the recip.
                emit_attnv_pair(prev, JC // 2 - 2)
                emit_attnv_pair(prev, JC // 2 - 1)
                po_drain(po_prev_q, 2)
                emit_den_pre(prev)
                emit_den_post(prev)
                prev = None
                po_drain(po_q, po_q.n)

    nc.finalize()
    return nc


def _get_built():
    global _built
    if _built is None:
        _built = build_bass()
    return _built


def _bf16(x):
    return np.ascontiguousarray(x.astype(BF16NP))


def _chunk_w(w):
    """[HID, M] -> [P, KC, M]  (hid = ko*P + ki -> [ki, ko, m])"""
    m = w.shape[1]
    return np.ascontiguousarray(
        w.reshape(KC, P, m).transpose(1, 0, 2).astype(BF16NP)
    )


def make_in_maps(hidden_states, Wq, Wk, Wv, Wo):
    cos_t, sin_t = rope_tables()
    cos_b = _bf16(cos_t)
    sin_b = _bf16(sin_t)
    ident = _bf16(np.eye(P, dtype=np.float32))
    ones_mat = _bf16(np.ones((P, P), np.float32))
    # x: [S, HID] -> [P(ki), NB, KC(ko), SB]
    xTs = []
    for b in range(B):
        hs = hidden_states[b].reshape(NB, SB, KC, P)
        xTs.append(np.ascontiguousarray(hs.transpose(3, 0, 2, 1).astype(BF16NP)))
    in_maps = []
    for core in range(N_CORES):
        b, g = divmod(core, NKV)
        wo_slice = Wo[g * GROUPS * HD : (g + 1) * GROUPS * HD, :]
        wo_r = wo_slice.reshape(GROUPS, HD, HID).transpose(1, 0, 2)
        in_maps.append(
            {
                "xT": xTs[b],
                "wq": _chunk_w(Wq[:, g * GROUPS * HD : (g + 1) * GROUPS * HD]),
                "wk": _chunk_w(Wk[:, g * HD : (g + 1) * HD]),
                "wv": _chunk_w(Wv[:, g * HD : (g + 1) * HD]),
                "wo": _bf16(wo_r),
                "cos_t": cos_b,
                "sin_t": sin_b,
                "ident": ident,
                "ones_mat": ones_mat,
            }
        )
    return in_maps


def kernel(hidden_states, Wq, Wk, Wv, Wo, trace=False):
    from concourse.bass_utils import run_bass_kernel_spmd

    hidden_states = np.asarray(hidden_states, dtype=np.float32)
    Wq = np.asarray(Wq, dtype=np.float32)
    Wk = np.asarray(Wk, dtype=np.float32)
    Wv = np.asarray(Wv, dtype=np.float32)
    Wo = np.asarray(Wo, dtype=np.float32)

    nc = _get_built()
    in_maps = make_in_maps(hidden_states, Wq, Wk, Wv, Wo)
    res = run_bass_kernel_spmd(nc, in_maps, core_ids=list(range(N_CORES)), trace=trace)

    out = np.zeros((B, S, HID), dtype=np.float32)
    for core in range(N_CORES):
        b = core // NKV
        out[b] += res.results[core]["out"].astype(np.float32)
    if trace:
        kernel.last_result = res
    return out

